# revision 10
# baseline (speedup 1.0000x reference)
"""Trainium2 Bass kernel for nn_Camada_33612414059004.

Computes, for x:[B,N,D,S], M:[N,N], w_syn:[N,D,S], b_dend:[N,D],
w_dend:[N,D], b_soma:[N]:

    xm    = einsum('bids,oi->bods', x, M)
    dend  = tanh(einsum('bnds,nds->bnd', xm, w_syn) + b_dend)
    soma  = einsum('bnd,nd->bn', dend, w_dend) + b_soma
    out   = sigmoid(soma)                                  # [B, N]

Sharding: data-parallel over batch across 8 NeuronCores (B=64 -> 8/core),
zero cross-core communication.  Per core the dominant work is the
connectivity matmul  M[o,i] @ x[i, (b,d,s)]  in fp8-e4m3 with
perf_mode=DoubleRow (the 0/1 connectivity matrix is exact in fp8; x
quantization costs ~0.5% final rel err vs the 2e-2 gate): 4 K=256
chunk-pairs x 8 o-tiles x 2 halves = 64 matmuls of N=512 at ~242ns warm.

Each o-tile accumulates into one [128, 1024] two-bank PSUM tile.
Postprocess per tile, balanced so every engine stays under the ~2.0us
PE pace:
  Scalar: one ACT-copy PSUM -> bf16 [128,1024] ((1024+352)/1.2), tanh,
          sigmoid(+b_soma per-partition bias).
  Vector: one bf16 2x w_syn multiply, s-reduce as bf16 pairwise tree
          (16->8->4 adds, then a 4->1 tensor_reduce), b_dend add on
          even tiles.
  GpSimd: b_dend add on odd tiles, soma stage (w_dend mult + d-tree),
          mt/params DMA issue.
The last tile skips the scalar copy and runs its multiplies directly
from PSUM on DVE (h0 early via h-outer matmul order) with the soma on
DVE, minimizing the serial tail after the final matmul.

Schedule: o-tiles 0-1 accumulate k-outer, riding the input DMA stream;
tiles 2-7 run k-inner so each tile's postprocess overlaps the next
tile's matmuls.  x chunk-pairs on the Sync HWDGE; the first mt
chunk-pair on Scalar (free early), the rest + params on GpSimd; the
first x/mt chunk-pair is split in column halves so the first real
matmul waits on only 2x128KB.
"""

import numpy as np
import ml_dtypes
from contextlib import ExitStack

import concourse.bass as bass
import concourse.mybir as mybir
import concourse.tile as tile

B, N, D, S = 64, 1024, 8, 16
NCORES = 8
BC = B // NCORES          # batches per core = 8
DS = D * S                # 128
P = 128                   # SBUF partitions
KT = N // P               # 8 contraction chunks (input neurons)
KT2 = KT // 2             # 4 DoubleRow chunk pairs (K=256 each)
OT = N // P               # 8 output-neuron tiles
FH = 512                  # one fp32 PSUM bank
BD = BC * D               # 64
GRP = 2                   # o-tiles in the k-outer leading group

F32 = mybir.dt.float32
BF16 = mybir.dt.bfloat16
FP8 = mybir.dt.float8e4

# packed fp32 params: b_dend | b_soma | w_syn_f32 (all o-tile-major)
PF_BD, PF_BS, PF_WS = 0, OT * D, OT * D + OT
PF_COLS = OT * D + OT + OT * DS          # 1096
# packed bf16 params: w_syn | w_dend
PB_WS, PB_WD = 0, OT * DS
PB_COLS = OT * DS + OT * D               # 1088

_NC_CACHE = {}


def legalize_waits(nc, max_attached=1):
    """Split multi-semaphore waits onto preceding same-engine NOPs.

    The walrus build in this environment accepts at most one sync-wait
    command per instruction (setupSyncWait: "Too many sync wait commands"),
    but Tile attaches one wait per out-of-date engine clock.  An engine is
    in-order, so hoisting the extra waits onto NOPs immediately before the
    instruction is semantics-preserving.
    """
    nid = 0
    for f in nc.m.functions:
        for blk in f.blocks:
            new = []
            changed = False
            for inst in blk.instructions:
                si = inst.sync_info
                if si is not None and si.on_wait and len(si.on_wait) > max_attached:
                    waits = list(si.on_wait)
                    for w in waits[:-max_attached]:
                        nid += 1
                        nop = mybir.InstNoOp(name=f"WSPLIT-{nid}", ins=[], outs=[])
                        nop.engine = inst.engine
                        nop.sync_info = mybir.SyncInfo(on_wait=[w], on_update=[])
                        new.append(nop)
                    inst.sync_info = mybir.SyncInfo(
                        on_wait=waits[-max_attached:], on_update=list(si.on_update)
                    )
                    changed = True
                new.append(inst)
            if changed:
                blk.instructions = new
    return nc


def build_nc(legalize=True):
    """Build the single-core Bass program (SPMD: same program on all cores)."""
    nc = bass.Bass()
    mt = nc.declare_dram_parameter("mt", [N, N], FP8, isOutput=False)
    xc = nc.declare_dram_parameter("xc", [N, BC * DS], FP8, isOutput=False)
    pf32 = nc.declare_dram_parameter("pf32", [P, PF_COLS], F32, isOutput=False)
    pbf = nc.declare_dram_parameter("pbf", [P, PB_COLS], BF16, isOutput=False)
    out = nc.declare_dram_parameter("out", [P, OT * BC], F32, isOutput=True)

    AF = mybir.ActivationFunctionType
    AX = mybir.AxisListType
    OP = mybir.AluOpType
    DR = mybir.MatmulPerfMode.DoubleRow

    with tile.TileContext(nc) as tc, ExitStack() as ctx:
        wpool = ctx.enter_context(tc.tile_pool(name="weights", bufs=1))
        xpool = ctx.enter_context(tc.tile_pool(name="xin", bufs=1))
        pspool = ctx.enter_context(tc.tile_pool(name="ps", bufs=4, space="PSUM"))
        prpool = ctx.enter_context(tc.tile_pool(name="prp", bufs=3))
        smpool = ctx.enter_context(tc.tile_pool(name="smp", bufs=3))

        # --- PE pre-warm: dummy matmuls on memset scratch while the first
        # input chunk is in flight, lifting the HAM clock gate (1.2 ->
        # 2.4 GHz needs ~3.4us of sustained PE activity).  The memset runs
        # on Vector (idle early) so GpSimd can start issuing DMAs. ---
        warm_sb = wpool.tile([P, FH], BF16, tag="warm", name="warm_sb")
        nc.vector.memset(warm_sb[:], 0.0)
        warm_ps = pspool.tile([P, 2 * FH], F32, tag="ps", name="warm_ps")
        for _ in range(7):
            nc.tensor.matmul(
                warm_ps[:, 0:FH], lhsT=warm_sb[:, 0:P], rhs=warm_sb[:],
                start=True, stop=True,
            )

        # --- input DMAs: per DoubleRow chunk-pair [128, (g=2, cols)] tiles
        # (contraction row g*128+p).  x on Sync; first mt pair on Scalar,
        # the rest + params on GpSimd.  The first chunk-pair is split in
        # column halves so the first matmul waits on only 2x128KB. ---
        x_tiles, mt_tiles = [], []
        x0h, mt0h = [], []
        for h in range(2):
            xt = xpool.tile([P, 2 * FH], FP8, tag=f"x0{h}", name=f"x0{h}")
            nc.sync.dma_start(
                xt[:].rearrange("p (g c) -> p g c", g=2),
                xc[0:2 * P, h * FH:(h + 1) * FH].rearrange(
                    "(g p) c -> p g c", g=2),
            )
            x0h.append(xt)
            mtk = xpool.tile([P, 2 * FH], FP8, tag=f"m0{h}", name=f"m0{h}")
            nc.scalar.dma_start(
                mtk[:].rearrange("p (g c) -> p g c", g=2),
                mt[0:2 * P, h * FH:(h + 1) * FH].rearrange(
                    "(g p) c -> p g c", g=2),
            )
            mt0h.append(mtk)
        x_tiles.append(None)
        mt_tiles.append(None)
        for k in range(1, KT2):
            xt = xpool.tile([P, 2 * BC * DS], FP8, tag=f"x{k}", name=f"x{k}")
            mtk = xpool.tile([P, 2 * N], FP8, tag=f"m{k}", name=f"m{k}")
            nc.sync.dma_start(
                xt[:].rearrange("p (g c) -> p g c", g=2),
                xc[k * 2 * P:(k + 1) * 2 * P, :].rearrange(
                    "(g p) c -> p g c", g=2),
            )
            nc.gpsimd.dma_start(
                mtk[:].rearrange("p (g c) -> p g c", g=2),
                mt[k * 2 * P:(k + 1) * 2 * P, :].rearrange(
                    "(g p) c -> p g c", g=2),
            )
            x_tiles.append(xt)
            mt_tiles.append(mtk)

        pbf_sb = wpool.tile([P, PB_COLS], BF16, tag="pbf", name="pbf_sb")
        nc.gpsimd.dma_start(pbf_sb[:], pbf[:, :])
        pf_sb = wpool.tile([P, PF_COLS], F32, tag="pf32", name="pf_sb")
        nc.gpsimd.dma_start(pf_sb[:], pf32[:, :])

        out_sb = wpool.tile([P, OT * BC], F32, tag="out", name="out_sb")

        def wsyn_bf(t):
            return (pbf_sb[:, PB_WS + t * DS:PB_WS + (t + 1) * DS]
                    .unsqueeze(1).broadcast_to([P, BC, DS]))

        def wsyn_f32(t):
            return (pf_sb[:, PF_WS + t * DS:PF_WS + (t + 1) * DS]
                    .unsqueeze(1).broadcast_to([P, BC, DS]))

        def bdend(t):
            return (pf_sb[:, PF_BD + t * D:PF_BD + (t + 1) * D]
                    .unsqueeze(1).broadcast_to([P, BC, D]))

        def wdend(t):
            return (pbf_sb[:, PB_WD + t * D:PB_WD + (t + 1) * D]
                    .unsqueeze(1).broadcast_to([P, BC, D]))

        def tree_and_bias(t, prod, bias_on_dve):
            # bf16 pairwise tree over s (16 -> 8 -> 4), then a 4->1
            # tensor_reduce, then the b_dend add.
            pv = prod[:].rearrange("p (bd s) -> p bd s", s=S)
            gr1 = smpool.tile([P, BD * 8], BF16, tag="gr1", name=f"gr1{t}")
            g1 = gr1[:].rearrange("p (bd s) -> p bd s", s=8)
            nc.vector.tensor_add(g1, pv[:, :, 0:8], pv[:, :, 8:16])
            gr2 = smpool.tile([P, BD * 4], BF16, tag="gr2", name=f"gr2{t}")
            g2 = gr2[:].rearrange("p (bd s) -> p bd s", s=4)
            nc.vector.tensor_add(g2, g1[:, :, 0:4], g1[:, :, 4:8])
            dp = smpool.tile([P, BD], F32, tag="dp", name=f"dp{t}")
            nc.vector.tensor_reduce(dp[:], g2, axis=AX.X, op=OP.add)
            eng = nc.vector if bias_on_dve else nc.gpsimd
            eng.tensor_add(
                dp[:].rearrange("p (b d) -> p b d", d=D),
                dp[:].rearrange("p (b d) -> p b d", d=D),
                bdend(t),
            )
            return dp

        def soma_stage(t, dend, on_dve):
            sp = smpool.tile([P, BD], BF16, tag="sp", name=f"sp{t}")
            spv = sp[:].rearrange("p (b d) -> p b d", d=D)
            soma = smpool.tile([P, BC], F32, tag="soma", name=f"soma{t}")
            if on_dve:
                nc.vector.tensor_mul(
                    spv, dend[:].rearrange("p (b d) -> p b d", d=D), wdend(t))
                nc.vector.tensor_reduce(soma[:], spv, axis=AX.X, op=OP.add)
            else:
                nc.gpsimd.tensor_mul(
                    spv, dend[:].rearrange("p (b d) -> p b d", d=D), wdend(t))
                r1 = smpool.tile([P, BC * 4], BF16, tag="r1", name=f"r1{t}")
                r1v = r1[:].rearrange("p (b d) -> p b d", d=4)
                nc.gpsimd.tensor_add(r1v, spv[:, :, 0:4], spv[:, :, 4:8])
                r2 = smpool.tile([P, BC * 2], BF16, tag="r2", name=f"r2{t}")
                r2v = r2[:].rearrange("p (b d) -> p b d", d=2)
                nc.gpsimd.tensor_add(r2v, r1v[:, :, 0:2], r1v[:, :, 2:4])
                nc.gpsimd.tensor_add(
                    soma[:].unsqueeze(2), r2v[:, :, 0:1], r2v[:, :, 1:2])
            nc.scalar.activation(
                out_sb[:, t * BC:(t + 1) * BC], soma[:], AF.Sigmoid,
                bias=pf_sb[:, PF_BS + t:PF_BS + t + 1],
            )

        def postprocess(t, pst):
            # Scalar: one PSUM -> SBUF bf16 copy over both banks; DVE: one
            # bf16 2x w_syn multiply, then the s-tree (same-engine chain).
            cp = prpool.tile([P, 2 * FH], BF16, tag="cp", name=f"cp{t}")
            nc.scalar.activation(cp[:], pst[:], AF.Copy)
            prod = prpool.tile([P, BC * DS], BF16, tag="prod", name=f"prod{t}")
            nc.vector.tensor_mul(
                prod[:].rearrange("p (b q) -> p b q", b=BC),
                cp[:].rearrange("p (b q) -> p b q", b=BC),
                wsyn_bf(t),
            )
            dp = tree_and_bias(t, prod, bias_on_dve=(t % 2 == 0))
            dend = smpool.tile([P, BD], BF16, tag="dend", name=f"dend{t}")
            nc.scalar.activation(dend[:], dp[:], AF.Tanh)
            soma_stage(t, dend, on_dve=False)

        def postprocess_last(t, pst):
            # Tail-latency variant: no scalar copy -- DVE multiplies both
            # halves straight from PSUM (h0 is ready ~1us early thanks to
            # the h-outer matmul order), soma on DVE, bias on DVE.
            prod = prpool.tile([P, BC * DS], BF16, tag="prod", name=f"prod{t}")
            for h in range(2):
                nc.vector.tensor_mul(
                    prod[:, h * FH:(h + 1) * FH].rearrange(
                        "p (b q) -> p b q", b=BC // 2),
                    pst[:, h * FH:(h + 1) * FH].rearrange(
                        "p (b q) -> p b q", b=BC // 2),
                    wsyn_f32(t)[:, h * (BC // 2):(h + 1) * (BC // 2), :],
                )
            dp = tree_and_bias(t, prod, bias_on_dve=True)
            dend = smpool.tile([P, BD], BF16, tag="dend", name=f"dend{t}")
            nc.scalar.activation(dend[:], dp[:], AF.Tanh)
            soma_stage(t, dend, on_dve=True)

        def mm(pst, t, k, h):
            if k == 0:
                src = x0h[h]
                rhs = src[:].rearrange("p (g c) -> p g c", g=2)[:, :, :]
                msrc = mt0h[t // 4]
                lhs = (msrc[:].rearrange("p (g c) -> p g c", g=2)
                       [:, :, (t % 4) * P:(t % 4 + 1) * P])
            else:
                rhs = (x_tiles[k][:].rearrange("p (g c) -> p g c", g=2)
                       [:, :, h * FH:(h + 1) * FH])
                lhs = (mt_tiles[k][:].rearrange("p (g c) -> p g c", g=2)
                       [:, :, t * P:(t + 1) * P])
            nc.tensor.matmul(
                pst[:, h * FH:(h + 1) * FH], lhsT=lhs, rhs=rhs,
                start=(k == 0), stop=(k == KT2 - 1), perf_mode=DR,
            )

        # Leading group: k-outer over o-tiles 0..GRP-1 — per-k PE work
        # paces with the chunk-pair DMA stream.
        pst = {}
        for t in range(GRP):
            pst[t] = pspool.tile([P, 2 * FH], F32, tag="ps", name=f"ps{t}")
        for k in range(KT2):
            for t in range(GRP):
                for h in range(2):
                    mm(pst[t], t, k, h)
        for t in range(GRP):
            postprocess(t, pst[t])

        # Trailing o-tiles: k-inner; each tile's chain overlaps the next
        # tile's matmuls.  The last tile runs h-outer so its first PSUM
        # half completes ~1us before its last matmul.
        for t in range(GRP, OT):
            pstt = pspool.tile([P, 2 * FH], F32, tag="ps", name=f"ps{t}")
            if t == OT - 1:
                for h in range(2):
                    for k in range(KT2):
                        mm(pstt, t, k, h)
                postprocess_last(t, pstt)
            else:
                for k in range(KT2):
                    for h in range(2):
                        mm(pstt, t, k, h)
                postprocess(t, pstt)

        nc.sync.dma_start(out[:, :], out_sb[:])

    if legalize:
        legalize_waits(nc)
    return nc


def get_nc():
    if "nc" not in _NC_CACHE:
        _NC_CACHE["nc"] = build_nc()
    return _NC_CACHE["nc"]


def pack_params(w_syn, b_dend, w_dend, b_soma):
    """Pack per-neuron parameters into the fp32 and bf16 SBUF layouts
    (each section o-tile-major: column block t holds o-tile t's rows)."""
    ws = np.asarray(w_syn, np.float32).reshape(OT, P, DS).transpose(1, 0, 2).reshape(P, OT * DS)
    bd = np.asarray(b_dend, np.float32).reshape(OT, P, D).transpose(1, 0, 2).reshape(P, OT * D)
    wd = np.asarray(w_dend, np.float32).reshape(OT, P, D).transpose(1, 0, 2).reshape(P, OT * D)
    bs = np.asarray(b_soma, np.float32).reshape(OT, P).T
    pf = np.ascontiguousarray(np.concatenate([bd, bs, ws], axis=1))
    pb = np.ascontiguousarray(
        np.concatenate([ws, wd], axis=1).astype(ml_dtypes.bfloat16))
    return pf, pb


def prepare_in_maps(x, matriz_conexao, w_syn, b_dend, w_dend, b_soma):
    x = np.asarray(x, dtype=np.float32)
    mt_np = np.ascontiguousarray(
        np.asarray(matriz_conexao, np.float32).T).astype(ml_dtypes.float8_e4m3)
    pf, pb = pack_params(w_syn, b_dend, w_dend, b_soma)
    xt = np.ascontiguousarray(x.transpose(1, 0, 2, 3).reshape(N, B, DS))
    in_maps = []
    for c in range(NCORES):
        xc_np = np.ascontiguousarray(
            xt[:, c * BC:(c + 1) * BC, :].reshape(N, BC * DS)
        ).astype(ml_dtypes.float8_e4m3)
        in_maps.append({"mt": mt_np, "xc": xc_np, "pf32": pf, "pbf": pb})
    return in_maps


def assemble_output(results):
    outs = []
    for c in range(NCORES):
        oc = np.asarray(results[c]["out"])          # [P, OT*BC] = (oi, (t, b))
        outs.append(oc.reshape(P, OT, BC).transpose(2, 1, 0).reshape(BC, N))
    return np.ascontiguousarray(np.concatenate(outs, axis=0).astype(np.float32))


def kernel(x, matriz_conexao, w_syn, b_dend, w_dend, b_soma):
    from concourse.bass_utils import run_bass_kernel_spmd
    in_maps = prepare_in_maps(x, matriz_conexao, w_syn, b_dend, w_dend, b_soma)
    nc = get_nc()
    res = run_bass_kernel_spmd(nc, in_maps, list(range(NCORES)))
    return assemble_output(res.results)


# revision 17
# speedup vs baseline: 1.0469x; 1.0469x over previous
"""Trainium2 Bass kernel for nn_Camada_33612414059004.

Computes, for x:[B,N,D,S], M:[N,N], w_syn:[N,D,S], b_dend:[N,D],
w_dend:[N,D], b_soma:[N]:

    xm    = einsum('bids,oi->bods', x, M)
    dend  = tanh(einsum('bnds,nds->bnd', xm, w_syn) + b_dend)
    soma  = einsum('bnd,nd->bn', dend, w_dend) + b_soma
    out   = sigmoid(soma)                                  # [B, N]

Sharding: data-parallel over batch across 8 NeuronCores (B=64 -> 8/core),
zero cross-core communication.  Per core the dominant work is the
connectivity matmul  M[o,i] @ x[i, (b,d,s)]  in fp8-e4m3 with
perf_mode=DoubleRow (the 0/1 connectivity matrix is exact in fp8; x
quantization costs ~0.5% final rel err vs the 2e-2 gate): 4 K=256
chunk-pairs x 8 o-tiles x 2 halves = 64 matmuls of N=512 at ~242ns warm.

Each o-tile accumulates into one [128, 1024] two-bank PSUM tile.
Postprocess per tile, balanced so every engine stays under the ~2.0us
PE pace:
  Scalar: one ACT-copy PSUM -> bf16 [128,1024] ((1024+352)/1.2), tanh,
          sigmoid(+b_soma per-partition bias).
  Vector: one bf16 2x w_syn multiply, s-reduce as bf16 pairwise tree
          (16->8->4 adds, then a 4->1 tensor_reduce), b_dend add on
          even tiles.
  GpSimd: b_dend add on odd tiles, soma stage (w_dend mult + d-tree),
          mt/params DMA issue.
The last tile skips the scalar copy and runs its multiplies directly
from PSUM on DVE (h0 early via h-outer matmul order) with the soma on
DVE, minimizing the serial tail after the final matmul.

Schedule: o-tiles 0-1 accumulate k-outer, riding the input DMA stream;
tiles 2-7 run k-inner so each tile's postprocess overlaps the next
tile's matmuls.  x chunk-pairs on the Sync HWDGE; the first mt
chunk-pair on Scalar (free early), the rest + params on GpSimd; the
first x/mt chunk-pair is split in column halves so the first real
matmul waits on only 2x128KB.
"""

import numpy as np
import ml_dtypes
from contextlib import ExitStack

import concourse.bass as bass
import concourse.mybir as mybir
import concourse.tile as tile

B, N, D, S = 64, 1024, 8, 16
NCORES = 8
BC = B // NCORES          # batches per core = 8
DS = D * S                # 128
P = 128                   # SBUF partitions
KT = N // P               # 8 contraction chunks (input neurons)
KT2 = KT // 2             # 4 DoubleRow chunk pairs (K=256 each)
OT = N // P               # 8 output-neuron tiles
FH = 512                  # one fp32 PSUM bank
BD = BC * D               # 64
GRP = 3                   # o-tiles in the k-outer leading group

F32 = mybir.dt.float32
BF16 = mybir.dt.bfloat16
FP8 = mybir.dt.float8e4

# packed fp32 params: b_dend | b_soma (all o-tile-major)
PF_BD, PF_BS = 0, OT * D
PF_COLS = OT * D + OT                    # 72
# packed bf16 params: w_syn | w_dend
PB_WS, PB_WD = 0, OT * DS
PB_COLS = OT * DS + OT * D               # 1088

_NC_CACHE = {}


def legalize_waits(nc, max_attached=1):
    """Split multi-semaphore waits onto preceding same-engine NOPs.

    The walrus build in this environment accepts at most one sync-wait
    command per instruction (setupSyncWait: "Too many sync wait commands"),
    but Tile attaches one wait per out-of-date engine clock.  An engine is
    in-order, so hoisting the extra waits onto NOPs immediately before the
    instruction is semantics-preserving.
    """
    nid = 0
    for f in nc.m.functions:
        for blk in f.blocks:
            new = []
            changed = False
            for inst in blk.instructions:
                si = inst.sync_info
                if si is not None and si.on_wait and len(si.on_wait) > max_attached:
                    waits = list(si.on_wait)
                    for w in waits[:-max_attached]:
                        nid += 1
                        nop = mybir.InstNoOp(name=f"WSPLIT-{nid}", ins=[], outs=[])
                        nop.engine = inst.engine
                        nop.sync_info = mybir.SyncInfo(on_wait=[w], on_update=[])
                        new.append(nop)
                    inst.sync_info = mybir.SyncInfo(
                        on_wait=waits[-max_attached:], on_update=list(si.on_update)
                    )
                    changed = True
                new.append(inst)
            if changed:
                blk.instructions = new
    return nc


def build_nc(legalize=True):
    """Build the single-core Bass program (SPMD: same program on all cores)."""
    nc = bass.Bass()
    mt = nc.declare_dram_parameter("mt", [N, N], FP8, isOutput=False)
    xc = nc.declare_dram_parameter("xc", [N, BC * DS], FP8, isOutput=False)
    pf32 = nc.declare_dram_parameter("pf32", [P, PF_COLS], F32, isOutput=False)
    pbf = nc.declare_dram_parameter("pbf", [P, PB_COLS], BF16, isOutput=False)
    out = nc.declare_dram_parameter("out", [P, OT * BC], F32, isOutput=True)

    AF = mybir.ActivationFunctionType
    AX = mybir.AxisListType
    OP = mybir.AluOpType
    DR = mybir.MatmulPerfMode.DoubleRow

    with tile.TileContext(nc) as tc, ExitStack() as ctx:
        wpool = ctx.enter_context(tc.tile_pool(name="weights", bufs=1))
        xpool = ctx.enter_context(tc.tile_pool(name="xin", bufs=1))
        pspool = ctx.enter_context(tc.tile_pool(name="ps", bufs=4, space="PSUM"))
        prpool = ctx.enter_context(tc.tile_pool(name="prp", bufs=3))
        smpool = ctx.enter_context(tc.tile_pool(name="smp", bufs=3))

        # --- PE pre-warm: dummy matmuls on memset scratch while the first
        # input chunk is in flight, lifting the HAM clock gate (1.2 ->
        # 2.4 GHz needs ~3.4us of sustained PE activity).  The memset runs
        # on Vector (idle early) so GpSimd can start issuing DMAs. ---
        warm_sb = wpool.tile([P, FH], BF16, tag="warm", name="warm_sb")
        nc.vector.memset(warm_sb[:], 0.0)
        warm_ps = pspool.tile([P, 2 * FH], F32, tag="ps", name="warm_ps")
        for _ in range(4):
            nc.tensor.matmul(
                warm_ps[:, 0:FH], lhsT=warm_sb[:, 0:P], rhs=warm_sb[:],
                start=True, stop=True,
            )

        # --- input DMAs: per DoubleRow chunk-pair [128, 2048] tiles.  The
        # host pre-interleaves rows (kp, p, g) so each partition's 2 KB is
        # contiguous in DRAM — maximal per-queue DMA rate.  x rides the
        # Sync HWDGE, mt the Scalar HWDGE, params the GpSimd SWDGE. ---
        x_tiles, mt_tiles = [], []
        for k in range(KT2):
            xt = xpool.tile([P, 2 * BC * DS], FP8, tag=f"x{k}", name=f"x{k}")
            mtk = xpool.tile([P, 2 * N], FP8, tag=f"m{k}", name=f"m{k}")
            nc.sync.dma_start(
                xt[:],
                xc[k * 2 * P:(k + 1) * 2 * P, :].rearrange(
                    "(p g) c -> p (g c)", g=2),
            )
            nc.scalar.dma_start(
                mtk[:],
                mt[k * 2 * P:(k + 1) * 2 * P, :].rearrange(
                    "(p g) c -> p (g c)", g=2),
            )
            x_tiles.append(xt)
            mt_tiles.append(mtk)

        pbf_sb = wpool.tile([P, PB_COLS], BF16, tag="pbf", name="pbf_sb")
        nc.gpsimd.dma_start(pbf_sb[:], pbf[:, :])
        pf_sb = wpool.tile([P, PF_COLS], F32, tag="pf32", name="pf_sb")
        nc.gpsimd.dma_start(pf_sb[:], pf32[:, :])

        out_sb = wpool.tile([P, OT * BC], F32, tag="out", name="out_sb")

        def wsyn_bf(t):
            return (pbf_sb[:, PB_WS + t * DS:PB_WS + (t + 1) * DS]
                    .unsqueeze(1).broadcast_to([P, BC, DS]))

        def bdend(t):
            return (pf_sb[:, PF_BD + t * D:PF_BD + (t + 1) * D]
                    .unsqueeze(1).broadcast_to([P, BC, D]))

        def wdend(t):
            return (pbf_sb[:, PB_WD + t * D:PB_WD + (t + 1) * D]
                    .unsqueeze(1).broadcast_to([P, BC, D]))

        def tree_and_bias(t, prod, bias_on_dve):
            # bf16 pairwise tree over s (16 -> 8 -> 4), then a 4->1
            # tensor_reduce, then the b_dend add.
            pv = prod[:].rearrange("p (bd s) -> p bd s", s=S)
            gr1 = smpool.tile([P, BD * 8], BF16, tag="gr1", name=f"gr1{t}")
            g1 = gr1[:].rearrange("p (bd s) -> p bd s", s=8)
            nc.vector.tensor_add(g1, pv[:, :, 0:8], pv[:, :, 8:16])
            gr2 = smpool.tile([P, BD * 4], BF16, tag="gr2", name=f"gr2{t}")
            g2 = gr2[:].rearrange("p (bd s) -> p bd s", s=4)
            nc.vector.tensor_add(g2, g1[:, :, 0:4], g1[:, :, 4:8])
            dp = smpool.tile([P, BD], F32, tag="dp", name=f"dp{t}")
            nc.vector.tensor_reduce(dp[:], g2, axis=AX.X, op=OP.add)
            eng = nc.vector if bias_on_dve else nc.gpsimd
            eng.tensor_add(
                dp[:].rearrange("p (b d) -> p b d", d=D),
                dp[:].rearrange("p (b d) -> p b d", d=D),
                bdend(t),
            )
            return dp

        def soma_stage(t, dend, on_dve):
            sp = smpool.tile([P, BD], BF16, tag="sp", name=f"sp{t}")
            spv = sp[:].rearrange("p (b d) -> p b d", d=D)
            soma = smpool.tile([P, BC], F32, tag="soma", name=f"soma{t}")
            if on_dve:
                nc.vector.tensor_mul(
                    spv, dend[:].rearrange("p (b d) -> p b d", d=D), wdend(t))
                nc.vector.tensor_reduce(soma[:], spv, axis=AX.X, op=OP.add)
            else:
                nc.gpsimd.tensor_mul(
                    spv, dend[:].rearrange("p (b d) -> p b d", d=D), wdend(t))
                r1 = smpool.tile([P, BC * 4], BF16, tag="r1", name=f"r1{t}")
                r1v = r1[:].rearrange("p (b d) -> p b d", d=4)
                nc.gpsimd.tensor_add(r1v, spv[:, :, 0:4], spv[:, :, 4:8])
                r2 = smpool.tile([P, BC * 2], BF16, tag="r2", name=f"r2{t}")
                r2v = r2[:].rearrange("p (b d) -> p b d", d=2)
                nc.gpsimd.tensor_add(r2v, r1v[:, :, 0:2], r1v[:, :, 2:4])
                nc.gpsimd.tensor_add(
                    soma[:].unsqueeze(2), r2v[:, :, 0:1], r2v[:, :, 1:2])
            nc.scalar.activation(
                out_sb[:, t * BC:(t + 1) * BC], soma[:], AF.Sigmoid,
                bias=pf_sb[:, PF_BS + t:PF_BS + t + 1],
            )

        def postprocess(t, pst):
            # Scalar: one PSUM -> SBUF bf16 copy over both banks; DVE: one
            # bf16 2x w_syn multiply, then the s-tree (same-engine chain).
            cp = prpool.tile([P, 2 * FH], BF16, tag="cp", name=f"cp{t}")
            nc.scalar.activation(cp[:], pst[:], AF.Copy)
            prod = prpool.tile([P, BC * DS], BF16, tag="prod", name=f"prod{t}")
            nc.vector.tensor_mul(
                prod[:].rearrange("p (b q) -> p b q", b=BC),
                cp[:].rearrange("p (b q) -> p b q", b=BC),
                wsyn_bf(t),
            )
            dp = tree_and_bias(t, prod, bias_on_dve=(t % 2 == 0))
            dend = smpool.tile([P, BD], BF16, tag="dend", name=f"dend{t}")
            nc.scalar.activation(dend[:], dp[:], AF.Tanh)
            soma_stage(t, dend, on_dve=False)

        def postprocess_last(t, pst):
            # Tail-latency variant: no scalar copy -- DVE multiplies both
            # halves straight from PSUM (h0 is ready ~1us early thanks to
            # the h-outer matmul order), soma on DVE, bias on DVE.
            prod = prpool.tile([P, BC * DS], BF16, tag="prod", name=f"prod{t}")
            for h in range(2):
                nc.vector.tensor_mul(
                    prod[:, h * FH:(h + 1) * FH].rearrange(
                        "p (b q) -> p b q", b=BC // 2),
                    pst[:, h * FH:(h + 1) * FH].rearrange(
                        "p (b q) -> p b q", b=BC // 2),
                    wsyn_bf(t)[:, h * (BC // 2):(h + 1) * (BC // 2), :],
                )
            dp = tree_and_bias(t, prod, bias_on_dve=True)
            dend = smpool.tile([P, BD], BF16, tag="dend", name=f"dend{t}")
            nc.scalar.activation(dend[:], dp[:], AF.Tanh)
            soma_stage(t, dend, on_dve=True)

        def mm(pst, t, k, h):
            rhs = (x_tiles[k][:].rearrange("p (g c) -> p g c", g=2)
                   [:, :, h * FH:(h + 1) * FH])
            lhs = (mt_tiles[k][:].rearrange("p (g c) -> p g c", g=2)
                   [:, :, t * P:(t + 1) * P])
            nc.tensor.matmul(
                pst[:, h * FH:(h + 1) * FH], lhsT=lhs, rhs=rhs,
                start=(k == 0), stop=(k == KT2 - 1), perf_mode=DR,
            )

        # Leading group: k-outer over o-tiles 0..GRP-1 — per-k PE work
        # paces with the chunk-pair DMA stream.
        pst = {}
        for t in range(GRP):
            pst[t] = pspool.tile([P, 2 * FH], F32, tag="ps", name=f"ps{t}")
        for k in range(KT2):
            for t in range(GRP):
                for h in range(2):
                    mm(pst[t], t, k, h)
        for t in range(GRP):
            postprocess(t, pst[t])

        # Trailing o-tiles: k-inner; each tile's chain overlaps the next
        # tile's matmuls.  The last tile runs h-outer so its first PSUM
        # half completes ~1us before its last matmul.
        for t in range(GRP, OT):
            pstt = pspool.tile([P, 2 * FH], F32, tag="ps", name=f"ps{t}")
            if t == OT - 1:
                for h in range(2):
                    for k in range(KT2):
                        mm(pstt, t, k, h)
                postprocess_last(t, pstt)
            else:
                for k in range(KT2):
                    for h in range(2):
                        mm(pstt, t, k, h)
                postprocess(t, pstt)

        nc.sync.dma_start(out[:, :], out_sb[:])

    if legalize:
        legalize_waits(nc)
    return nc


def get_nc():
    if "nc" not in _NC_CACHE:
        _NC_CACHE["nc"] = build_nc()
    return _NC_CACHE["nc"]


def pack_params(w_syn, b_dend, w_dend, b_soma):
    """Pack per-neuron parameters into the fp32 and bf16 SBUF layouts
    (each section o-tile-major: column block t holds o-tile t's rows)."""
    ws = np.asarray(w_syn, np.float32).reshape(OT, P, DS).transpose(1, 0, 2).reshape(P, OT * DS)
    bd = np.asarray(b_dend, np.float32).reshape(OT, P, D).transpose(1, 0, 2).reshape(P, OT * D)
    wd = np.asarray(w_dend, np.float32).reshape(OT, P, D).transpose(1, 0, 2).reshape(P, OT * D)
    bs = np.asarray(b_soma, np.float32).reshape(OT, P).T
    pf = np.ascontiguousarray(np.concatenate([bd, bs], axis=1))
    pb = np.ascontiguousarray(
        np.concatenate([ws, wd], axis=1).astype(ml_dtypes.bfloat16))
    return pf, pb


def interleave_rows(a):
    """Reorder [N, C] rows from (kp, g, p) to (kp, p, g) so each SBUF
    partition's DoubleRow pair is one contiguous 2C-byte DRAM run."""
    return np.ascontiguousarray(
        a.reshape(KT2, 2, P, a.shape[1]).transpose(0, 2, 1, 3)
        .reshape(N, a.shape[1]))


def prepare_in_maps(x, matriz_conexao, w_syn, b_dend, w_dend, b_soma):
    x = np.asarray(x, dtype=np.float32)
    mt_np = interleave_rows(
        np.asarray(matriz_conexao, np.float32).T.astype(ml_dtypes.float8_e4m3))
    pf, pb = pack_params(w_syn, b_dend, w_dend, b_soma)
    xt = np.ascontiguousarray(x.transpose(1, 0, 2, 3).reshape(N, B, DS))
    in_maps = []
    for c in range(NCORES):
        xc_np = interleave_rows(
            xt[:, c * BC:(c + 1) * BC, :].reshape(N, BC * DS)
            .astype(ml_dtypes.float8_e4m3))
        in_maps.append({"mt": mt_np, "xc": xc_np, "pf32": pf, "pbf": pb})
    return in_maps


def assemble_output(results):
    outs = []
    for c in range(NCORES):
        oc = np.asarray(results[c]["out"])          # [P, OT*BC] = (oi, (t, b))
        outs.append(oc.reshape(P, OT, BC).transpose(2, 1, 0).reshape(BC, N))
    return np.ascontiguousarray(np.concatenate(outs, axis=0).astype(np.float32))


def kernel(x, matriz_conexao, w_syn, b_dend, w_dend, b_soma):
    from concourse.bass_utils import run_bass_kernel_spmd
    in_maps = prepare_in_maps(x, matriz_conexao, w_syn, b_dend, w_dend, b_soma)
    nc = get_nc()
    res = run_bass_kernel_spmd(nc, in_maps, list(range(NCORES)))
    return assemble_output(res.results)


# revision 22
# speedup vs baseline: 1.0868x; 1.0382x over previous
"""Trainium2 Bass kernel for nn_Camada_33612414059004.

Computes, for x:[B,N,D,S], M:[N,N], w_syn:[N,D,S], b_dend:[N,D],
w_dend:[N,D], b_soma:[N]:

    xm    = einsum('bids,oi->bods', x, M)
    dend  = tanh(einsum('bnds,nds->bnd', xm, w_syn) + b_dend)
    soma  = einsum('bnd,nd->bn', dend, w_dend) + b_soma
    out   = sigmoid(soma)                                  # [B, N]

Sharding: data-parallel over batch across 8 NeuronCores (B=64 -> 8/core),
zero cross-core communication.  Per core the dominant work is the
connectivity matmul  M[o,i] @ x[i, (b,d,s)]  in fp8-e4m3 with
perf_mode=DoubleRow (the 0/1 connectivity matrix is exact in fp8; x
quantization costs ~0.5% final rel err vs the 2e-2 gate): 4 K=256
chunk-pairs x 8 o-tiles x 2 halves = 64 matmuls of N=512 at ~242ns warm.

Each o-tile accumulates into one [128, 1024] two-bank PSUM tile.
Postprocess per tile, balanced so every engine stays under the ~2.0us
PE pace:
  Scalar: one ACT-copy PSUM -> bf16 [128,1024] ((1024+352)/1.2), tanh,
          sigmoid(+b_soma per-partition bias).
  Vector: one bf16 2x w_syn multiply, s-reduce as bf16 pairwise tree
          (16->8->4 adds, then a 4->1 tensor_reduce), b_dend add on
          even tiles.
  GpSimd: b_dend add on odd tiles, soma stage (w_dend mult + d-tree),
          mt/params DMA issue.
The last tile skips the scalar copy and runs its multiplies directly
from PSUM on DVE (h0 early via h-outer matmul order) with the soma on
DVE, minimizing the serial tail after the final matmul.

Schedule: o-tiles 0-1 accumulate k-outer, riding the input DMA stream;
tiles 2-7 run k-inner so each tile's postprocess overlaps the next
tile's matmuls.  x chunk-pairs on the Sync HWDGE; the first mt
chunk-pair on Scalar (free early), the rest + params on GpSimd; the
first x/mt chunk-pair is split in column halves so the first real
matmul waits on only 2x128KB.
"""

import numpy as np
import ml_dtypes
from contextlib import ExitStack

import concourse.bass as bass
import concourse.mybir as mybir
import concourse.tile as tile

B, N, D, S = 64, 1024, 8, 16
NCORES = 8
BC = B // NCORES          # batches per core = 8
DS = D * S                # 128
P = 128                   # SBUF partitions
KT = N // P               # 8 contraction chunks (input neurons)
KT2 = KT // 2             # 4 DoubleRow chunk pairs (K=256 each)
OT = N // P               # 8 output-neuron tiles
FH = 512                  # one fp32 PSUM bank
BD = BC * D               # 64
GRP = 2                   # o-tiles in the k-outer leading group

F32 = mybir.dt.float32
BF16 = mybir.dt.bfloat16
FP8 = mybir.dt.float8e4

# packed fp32 params: b_dend | b_soma (all o-tile-major)
PF_BD, PF_BS = 0, OT * D
PF_COLS = OT * D + OT                    # 72
# packed bf16 params: w_syn | w_dend
PB_WS, PB_WD = 0, OT * DS
PB_COLS = OT * DS + OT * D               # 1088

_NC_CACHE = {}


def legalize_waits(nc, max_attached=1):
    """Split multi-semaphore waits onto preceding same-engine NOPs.

    The walrus build in this environment accepts at most one sync-wait
    command per instruction (setupSyncWait: "Too many sync wait commands"),
    but Tile attaches one wait per out-of-date engine clock.  An engine is
    in-order, so hoisting the extra waits onto NOPs immediately before the
    instruction is semantics-preserving.
    """
    nid = 0
    for f in nc.m.functions:
        for blk in f.blocks:
            new = []
            changed = False
            for inst in blk.instructions:
                si = inst.sync_info
                if si is not None and si.on_wait and len(si.on_wait) > max_attached:
                    waits = list(si.on_wait)
                    for w in waits[:-max_attached]:
                        nid += 1
                        nop = mybir.InstNoOp(name=f"WSPLIT-{nid}", ins=[], outs=[])
                        nop.engine = inst.engine
                        nop.sync_info = mybir.SyncInfo(on_wait=[w], on_update=[])
                        new.append(nop)
                    inst.sync_info = mybir.SyncInfo(
                        on_wait=waits[-max_attached:], on_update=list(si.on_update)
                    )
                    changed = True
                new.append(inst)
            if changed:
                blk.instructions = new
    return nc


def build_nc(legalize=True):
    """Build the single-core Bass program (SPMD: same program on all cores)."""
    nc = bass.Bass()
    mt = nc.declare_dram_parameter("mt", [N, N], FP8, isOutput=False)
    xc = nc.declare_dram_parameter("xc", [N, BC * DS], FP8, isOutput=False)
    pf32 = nc.declare_dram_parameter("pf32", [P, PF_COLS], F32, isOutput=False)
    pbf = nc.declare_dram_parameter("pbf", [P, PB_COLS], BF16, isOutput=False)
    out = nc.declare_dram_parameter("out", [P, OT * BC], F32, isOutput=True)

    AF = mybir.ActivationFunctionType
    AX = mybir.AxisListType
    OP = mybir.AluOpType
    DR = mybir.MatmulPerfMode.DoubleRow

    with tile.TileContext(nc) as tc, ExitStack() as ctx:
        wpool = ctx.enter_context(tc.tile_pool(name="weights", bufs=1))
        xpool = ctx.enter_context(tc.tile_pool(name="xin", bufs=1))
        pspool = ctx.enter_context(tc.tile_pool(name="ps", bufs=4, space="PSUM"))
        prpool = ctx.enter_context(tc.tile_pool(name="prp", bufs=3))
        smpool = ctx.enter_context(tc.tile_pool(name="smp", bufs=3))

        # --- PE pre-warm: dummy matmuls on memset scratch while the first
        # input chunk is in flight, lifting the HAM clock gate (1.2 ->
        # 2.4 GHz needs ~3.4us of sustained PE activity).  The memset runs
        # on Vector (idle early) so GpSimd can start issuing DMAs. ---
        warm_sb = wpool.tile([P, FH], BF16, tag="warm", name="warm_sb")
        nc.vector.memset(warm_sb[:], 0.0)
        warm_ps = pspool.tile([P, 2 * FH], F32, tag="ps", name="warm_ps")
        for _ in range(5):
            nc.tensor.matmul(
                warm_ps[:, 0:FH], lhsT=warm_sb[:, 0:P], rhs=warm_sb[:],
                start=True, stop=True,
            )

        # --- input DMAs.  x: per chunk-pair [128, 2048] tiles, rows
        # host-interleaved (kp, p, g) so each partition's 2 KB is one
        # contiguous DRAM run.  mt: host-repacked per o-tile so tile t's
        # weights for all chunk-pairs are one [128, 1024] contiguous
        # block.  The two HWDGE queues (Sync, Scalar) share ~270 GB/s, so
        # the interleave below lands mtt0/mtt1 + all of x first (the
        # postprocess of every tile is gated on x completing), then the
        # later tiles' mt blocks.  Params ride the GpSimd SWDGE. ---
        x_tiles = [None] * KT2
        mtt_tiles = [None] * OT

        def xdma(eng, k):
            xt = xpool.tile([P, 2 * BC * DS], FP8, tag=f"x{k}", name=f"x{k}")
            eng.dma_start(
                xt[:],
                xc[k * 2 * P:(k + 1) * 2 * P, :].rearrange(
                    "(p g) c -> p (g c)", g=2),
            )
            x_tiles[k] = xt

        def mdma(eng, t):
            mtk = xpool.tile([P, KT2 * 2 * P], FP8, tag=f"mtt{t}", name=f"mtt{t}")
            eng.dma_start(mtk[:], mt[t * P:(t + 1) * P, :])
            mtt_tiles[t] = mtk

        mdma(nc.sync, 0)
        mdma(nc.scalar, 1)
        xdma(nc.sync, 0)
        xdma(nc.scalar, 1)
        xdma(nc.sync, 2)
        xdma(nc.scalar, 3)
        mdma(nc.sync, 2)
        mdma(nc.scalar, 3)
        mdma(nc.sync, 4)
        mdma(nc.scalar, 5)
        mdma(nc.sync, 6)
        mdma(nc.scalar, 7)

        pbf_sb = wpool.tile([P, PB_COLS], BF16, tag="pbf", name="pbf_sb")
        nc.gpsimd.dma_start(pbf_sb[:], pbf[:, :])
        pf_sb = wpool.tile([P, PF_COLS], F32, tag="pf32", name="pf_sb")
        nc.gpsimd.dma_start(pf_sb[:], pf32[:, :])

        out_sb = wpool.tile([P, OT * BC], F32, tag="out", name="out_sb")

        def wsyn_bf(t):
            return (pbf_sb[:, PB_WS + t * DS:PB_WS + (t + 1) * DS]
                    .unsqueeze(1).broadcast_to([P, BC, DS]))

        def bdend(t):
            return (pf_sb[:, PF_BD + t * D:PF_BD + (t + 1) * D]
                    .unsqueeze(1).broadcast_to([P, BC, D]))

        def wdend(t):
            return (pbf_sb[:, PB_WD + t * D:PB_WD + (t + 1) * D]
                    .unsqueeze(1).broadcast_to([P, BC, D]))

        def postprocess_pair(te, ps_a, ps_b):
            # Two o-tiles (te, te+1) through one chain of double-width ops:
            # halves the per-op fixed cost and the cross-engine sync count.
            # Scalar: two PSUM -> SBUF bf16 copies into one [128, 2048]
            # tile; DVE: one bf16 2x w_syn multiply, s-tree, bias; GpSimd:
            # soma; Scalar: tanh + per-tile sigmoids.
            cp = prpool.tile([P, 4 * FH], BF16, tag="cp", name=f"cp{te}")
            nc.scalar.activation(cp[:, 0:2 * FH], ps_a[:], AF.Copy)
            nc.scalar.activation(cp[:, 2 * FH:4 * FH], ps_b[:], AF.Copy)
            prod = prpool.tile([P, 4 * FH], BF16, tag="prod", name=f"prod{te}")
            wsyn2 = (pbf_sb[:, PB_WS + te * DS:PB_WS + (te + 2) * DS]
                     .rearrange("p (u q) -> p u q", u=2).unsqueeze(2)
                     .broadcast_to([P, 2, BC, DS]))
            nc.vector.tensor_mul(
                prod[:].rearrange("p (u b q) -> p u b q", u=2, b=BC),
                cp[:].rearrange("p (u b q) -> p u b q", u=2, b=BC),
                wsyn2,
            )
            pv = prod[:].rearrange("p (bd s) -> p bd s", s=S)
            gr1 = smpool.tile([P, 2 * BD * 8], BF16, tag="gr1", name=f"gr1{te}")
            g1 = gr1[:].rearrange("p (bd s) -> p bd s", s=8)
            nc.vector.tensor_add(g1, pv[:, :, 0:8], pv[:, :, 8:16])
            gr2 = smpool.tile([P, 2 * BD * 4], BF16, tag="gr2", name=f"gr2{te}")
            g2 = gr2[:].rearrange("p (bd s) -> p bd s", s=4)
            nc.vector.tensor_add(g2, g1[:, :, 0:4], g1[:, :, 4:8])
            dp = smpool.tile([P, 2 * BD], F32, tag="dp", name=f"dp{te}")
            nc.vector.tensor_reduce(dp[:], g2, axis=AX.X, op=OP.add)
            bd2 = (pf_sb[:, PF_BD + te * D:PF_BD + (te + 2) * D]
                   .rearrange("p (u d) -> p u d", u=2).unsqueeze(2)
                   .broadcast_to([P, 2, BC, D]))
            dpv = dp[:].rearrange("p (u b d) -> p u b d", u=2, d=D)
            nc.vector.tensor_add(dpv, dpv, bd2)
            dend = smpool.tile([P, 2 * BD], BF16, tag="dend", name=f"dend{te}")
            nc.scalar.activation(dend[:], dp[:], AF.Tanh)
            wd2 = (pbf_sb[:, PB_WD + te * D:PB_WD + (te + 2) * D]
                   .rearrange("p (u d) -> p u d", u=2).unsqueeze(2)
                   .broadcast_to([P, 2, BC, D]))
            sp = smpool.tile([P, 2 * BD], BF16, tag="sp", name=f"sp{te}")
            spv = sp[:].rearrange("p (u b d) -> p u b d", u=2, d=D)
            nc.gpsimd.tensor_mul(
                spv, dend[:].rearrange("p (u b d) -> p u b d", u=2, d=D), wd2)
            r1 = smpool.tile([P, 2 * BC * 4], BF16, tag="r1", name=f"r1{te}")
            r1v = r1[:].rearrange("p (u b d) -> p u b d", u=2, d=4)
            nc.gpsimd.tensor_add(r1v, spv[:, :, :, 0:4], spv[:, :, :, 4:8])
            r2 = smpool.tile([P, 2 * BC * 2], BF16, tag="r2", name=f"r2{te}")
            r2v = r2[:].rearrange("p (u b d) -> p u b d", u=2, d=2)
            nc.gpsimd.tensor_add(r2v, r1v[:, :, :, 0:2], r1v[:, :, :, 2:4])
            soma = smpool.tile([P, 2 * BC], F32, tag="soma", name=f"soma{te}")
            nc.gpsimd.tensor_add(
                soma[:].rearrange("p (u b) -> p u b", u=2).unsqueeze(3),
                r2v[:, :, :, 0:1], r2v[:, :, :, 1:2])
            for u in range(2):
                nc.scalar.activation(
                    out_sb[:, (te + u) * BC:(te + u + 1) * BC],
                    soma[:, u * BC:(u + 1) * BC], AF.Sigmoid,
                    bias=pf_sb[:, PF_BS + te + u:PF_BS + te + u + 1],
                )

        def postprocess_single(t, pst):
            # Tail-latency variant for the last tiles: no scalar copy (DVE
            # multiplies straight from PSUM), soma + bias on DVE.
            prod = prpool.tile([P, BC * DS], BF16, tag="prods", name=f"prod{t}")
            nc.vector.tensor_mul(
                prod[:].rearrange("p (b q) -> p b q", b=BC),
                pst[:].rearrange("p (b q) -> p b q", b=BC),
                wsyn_bf(t),
            )
            pv = prod[:].rearrange("p (bd s) -> p bd s", s=S)
            gr1 = smpool.tile([P, BD * 8], BF16, tag="sg1", name=f"sg1{t}")
            g1 = gr1[:].rearrange("p (bd s) -> p bd s", s=8)
            nc.vector.tensor_add(g1, pv[:, :, 0:8], pv[:, :, 8:16])
            gr2 = smpool.tile([P, BD * 4], BF16, tag="sg2", name=f"sg2{t}")
            g2 = gr2[:].rearrange("p (bd s) -> p bd s", s=4)
            nc.vector.tensor_add(g2, g1[:, :, 0:4], g1[:, :, 4:8])
            dp = smpool.tile([P, BD], F32, tag="sdp", name=f"sdp{t}")
            nc.vector.tensor_reduce(dp[:], g2, axis=AX.X, op=OP.add)
            dpv = dp[:].rearrange("p (b d) -> p b d", d=D)
            nc.vector.tensor_add(dpv, dpv, bdend(t))
            dend = smpool.tile([P, BD], BF16, tag="sdd", name=f"sdd{t}")
            nc.scalar.activation(dend[:], dp[:], AF.Tanh)
            sp = smpool.tile([P, BD], BF16, tag="ssp", name=f"ssp{t}")
            spv = sp[:].rearrange("p (b d) -> p b d", d=D)
            nc.vector.tensor_mul(
                spv, dend[:].rearrange("p (b d) -> p b d", d=D), wdend(t))
            soma = smpool.tile([P, BC], F32, tag="ssm", name=f"ssm{t}")
            nc.vector.tensor_reduce(soma[:], spv, axis=AX.X, op=OP.add)
            nc.scalar.activation(
                out_sb[:, t * BC:(t + 1) * BC], soma[:], AF.Sigmoid,
                bias=pf_sb[:, PF_BS + t:PF_BS + t + 1],
            )

        def mm(pst, t, k, h):
            rhs = (x_tiles[k][:].rearrange("p (g c) -> p g c", g=2)
                   [:, :, h * FH:(h + 1) * FH])
            lhs = (mtt_tiles[t][:].rearrange("p (kp g c) -> p kp g c",
                                             kp=KT2, g=2)[:, k, :, :])
            nc.tensor.matmul(
                pst[:, h * FH:(h + 1) * FH], lhsT=lhs, rhs=rhs,
                start=(k == 0), stop=(k == KT2 - 1), perf_mode=DR,
            )

        # Leading group: k-outer over o-tiles {0,1} — per-k PE work paces
        # with the x chunk-pair DMA stream.
        pst = {}
        for t in range(OT):
            pst[t] = None
        for t in range(GRP):
            pst[t] = pspool.tile([P, 2 * FH], F32, tag="ps", name=f"ps{t}")
        for k in range(KT2):
            for t in range(GRP):
                for h in range(2):
                    mm(pst[t], t, k, h)
        postprocess_pair(0, pst[0], pst[1])

        # Trailing o-tiles: k-inner; pairs {2,3} and {4,5} overlap the
        # next tiles' matmuls; tiles 6 and 7 run as latency-optimized
        # singles to keep the serial tail short.
        for t in range(GRP, OT):
            pst[t] = pspool.tile([P, 2 * FH], F32, tag="ps", name=f"ps{t}")
            for k in range(KT2):
                for h in range(2):
                    mm(pst[t], t, k, h)
            if t in (3, 5):
                postprocess_pair(t - 1, pst[t - 1], pst[t])
            elif t in (6, 7):
                postprocess_single(t, pst[t])

        nc.sync.dma_start(out[:, :], out_sb[:])

    if legalize:
        legalize_waits(nc)
    return nc


def get_nc():
    if "nc" not in _NC_CACHE:
        _NC_CACHE["nc"] = build_nc()
    return _NC_CACHE["nc"]


def pack_params(w_syn, b_dend, w_dend, b_soma):
    """Pack per-neuron parameters into the fp32 and bf16 SBUF layouts
    (each section o-tile-major: column block t holds o-tile t's rows)."""
    ws = np.asarray(w_syn, np.float32).reshape(OT, P, DS).transpose(1, 0, 2).reshape(P, OT * DS)
    bd = np.asarray(b_dend, np.float32).reshape(OT, P, D).transpose(1, 0, 2).reshape(P, OT * D)
    wd = np.asarray(w_dend, np.float32).reshape(OT, P, D).transpose(1, 0, 2).reshape(P, OT * D)
    bs = np.asarray(b_soma, np.float32).reshape(OT, P).T
    pf = np.ascontiguousarray(np.concatenate([bd, bs], axis=1))
    pb = np.ascontiguousarray(
        np.concatenate([ws, wd], axis=1).astype(ml_dtypes.bfloat16))
    return pf, pb


def interleave_rows(a):
    """Reorder [N, C] rows from (kp, g, p) to (kp, p, g) so each SBUF
    partition's DoubleRow pair is one contiguous 2C-byte DRAM run."""
    return np.ascontiguousarray(
        a.reshape(KT2, 2, P, a.shape[1]).transpose(0, 2, 1, 3)
        .reshape(N, a.shape[1]))


def prepare_in_maps(x, matriz_conexao, w_syn, b_dend, w_dend, b_soma):
    x = np.asarray(x, dtype=np.float32)
    # mt repacked per o-tile: row (t, p), cols (kp, g, c) so tile t's
    # DoubleRow weights for all chunk-pairs are one contiguous block.
    mtT = np.asarray(matriz_conexao, np.float32).T.astype(ml_dtypes.float8_e4m3)
    mt_np = np.ascontiguousarray(
        mtT.reshape(KT2, 2, P, OT, P).transpose(3, 2, 0, 1, 4).reshape(N, N))
    pf, pb = pack_params(w_syn, b_dend, w_dend, b_soma)
    xt = np.ascontiguousarray(x.transpose(1, 0, 2, 3).reshape(N, B, DS))
    in_maps = []
    for c in range(NCORES):
        xc_np = interleave_rows(
            xt[:, c * BC:(c + 1) * BC, :].reshape(N, BC * DS)
            .astype(ml_dtypes.float8_e4m3))
        in_maps.append({"mt": mt_np, "xc": xc_np, "pf32": pf, "pbf": pb})
    return in_maps


def assemble_output(results):
    outs = []
    for c in range(NCORES):
        oc = np.asarray(results[c]["out"])          # [P, OT*BC] = (oi, (t, b))
        outs.append(oc.reshape(P, OT, BC).transpose(2, 1, 0).reshape(BC, N))
    return np.ascontiguousarray(np.concatenate(outs, axis=0).astype(np.float32))


def kernel(x, matriz_conexao, w_syn, b_dend, w_dend, b_soma):
    from concourse.bass_utils import run_bass_kernel_spmd
    in_maps = prepare_in_maps(x, matriz_conexao, w_syn, b_dend, w_dend, b_soma)
    nc = get_nc()
    res = run_bass_kernel_spmd(nc, in_maps, list(range(NCORES)))
    return assemble_output(res.results)


# revision 26
# speedup vs baseline: 1.1424x; 1.0512x over previous
"""Trainium2 Bass kernel for nn_Camada_33612414059004.

Computes, for x:[B,N,D,S], M:[N,N], w_syn:[N,D,S], b_dend:[N,D],
w_dend:[N,D], b_soma:[N]:

    xm    = einsum('bids,oi->bods', x, M)
    dend  = tanh(einsum('bnds,nds->bnd', xm, w_syn) + b_dend)
    soma  = einsum('bnd,nd->bn', dend, w_dend) + b_soma
    out   = sigmoid(soma)                                  # [B, N]

Sharding: data-parallel over batch across 8 NeuronCores (B=64 -> 8/core),
zero cross-core communication.  Per core the dominant work is the
connectivity matmul  M[o,i] @ x[i, (b,d,s)]  in fp8-e4m3 with
perf_mode=DoubleRow (the 0/1 connectivity matrix is exact in fp8; x
quantization costs ~0.5% final rel err vs the 2e-2 gate): 4 K=256
chunk-pairs x 8 o-tiles x 2 halves = 64 matmuls of N=512 at ~242ns warm.

Each o-tile accumulates into one [128, 1024] two-bank PSUM tile.
Postprocess per tile, balanced so every engine stays under the ~2.0us
PE pace:
  Scalar: one ACT-copy PSUM -> bf16 [128,1024] ((1024+352)/1.2), tanh,
          sigmoid(+b_soma per-partition bias).
  Vector: one bf16 2x w_syn multiply, s-reduce as bf16 pairwise tree
          (16->8->4 adds, then a 4->1 tensor_reduce), b_dend add on
          even tiles.
  GpSimd: b_dend add on odd tiles, soma stage (w_dend mult + d-tree),
          mt/params DMA issue.
The last tile skips the scalar copy and runs its multiplies directly
from PSUM on DVE (h0 early via h-outer matmul order) with the soma on
DVE, minimizing the serial tail after the final matmul.

Schedule: o-tiles 0-1 accumulate k-outer, riding the input DMA stream;
tiles 2-7 run k-inner so each tile's postprocess overlaps the next
tile's matmuls.  x chunk-pairs on the Sync HWDGE; the first mt
chunk-pair on Scalar (free early), the rest + params on GpSimd; the
first x/mt chunk-pair is split in column halves so the first real
matmul waits on only 2x128KB.
"""

import numpy as np
import ml_dtypes
from contextlib import ExitStack

import concourse.bass as bass
import concourse.mybir as mybir
import concourse.tile as tile

B, N, D, S = 64, 1024, 8, 16
NCORES = 8
BC = B // NCORES          # batches per core = 8
DS = D * S                # 128
P = 128                   # SBUF partitions
KT = N // P               # 8 contraction chunks (input neurons)
KT2 = KT // 2             # 4 DoubleRow chunk pairs (K=256 each)
OT = N // P               # 8 output-neuron tiles
FH = 512                  # one fp32 PSUM bank
BD = BC * D               # 64
GRP = 2                   # o-tiles in the k-outer leading group

F32 = mybir.dt.float32
BF16 = mybir.dt.bfloat16
FP8 = mybir.dt.float8e4

# packed fp32 params: b_dend | b_soma (all o-tile-major)
PF_BD, PF_BS = 0, OT * D
PF_COLS = OT * D + OT                    # 72
# packed bf16 params: w_syn | w_dend
PB_WS, PB_WD = 0, OT * DS
PB_COLS = OT * DS + OT * D               # 1088

_NC_CACHE = {}


def legalize_waits(nc, max_attached=1):
    """Split multi-semaphore waits onto preceding same-engine NOPs.

    The walrus build in this environment accepts at most one sync-wait
    command per instruction (setupSyncWait: "Too many sync wait commands"),
    but Tile attaches one wait per out-of-date engine clock.  An engine is
    in-order, so hoisting the extra waits onto NOPs immediately before the
    instruction is semantics-preserving.
    """
    nid = 0
    for f in nc.m.functions:
        for blk in f.blocks:
            new = []
            changed = False
            for inst in blk.instructions:
                si = inst.sync_info
                if si is not None and si.on_wait and len(si.on_wait) > max_attached:
                    waits = list(si.on_wait)
                    for w in waits[:-max_attached]:
                        nid += 1
                        nop = mybir.InstNoOp(name=f"WSPLIT-{nid}", ins=[], outs=[])
                        nop.engine = inst.engine
                        nop.sync_info = mybir.SyncInfo(on_wait=[w], on_update=[])
                        new.append(nop)
                    inst.sync_info = mybir.SyncInfo(
                        on_wait=waits[-max_attached:], on_update=list(si.on_update)
                    )
                    changed = True
                new.append(inst)
            if changed:
                blk.instructions = new
    return nc


def build_nc(legalize=True):
    """Build the single-core Bass program (SPMD: same program on all cores)."""
    nc = bass.Bass()
    mt = nc.declare_dram_parameter("mt", [N, N], FP8, isOutput=False)
    xc = nc.declare_dram_parameter("xc", [N, BC * DS], FP8, isOutput=False)
    pf32 = nc.declare_dram_parameter("pf32", [P, PF_COLS], F32, isOutput=False)
    pbf = nc.declare_dram_parameter("pbf", [P, PB_COLS], BF16, isOutput=False)
    out = nc.declare_dram_parameter("out", [P, OT * BC], F32, isOutput=True)

    AF = mybir.ActivationFunctionType
    AX = mybir.AxisListType
    OP = mybir.AluOpType
    DR = mybir.MatmulPerfMode.DoubleRow

    with tile.TileContext(nc) as tc, ExitStack() as ctx:
        wpool = ctx.enter_context(tc.tile_pool(name="weights", bufs=1))
        xpool = ctx.enter_context(tc.tile_pool(name="xin", bufs=1))
        pspool = ctx.enter_context(tc.tile_pool(name="ps", bufs=4, space="PSUM"))
        prpool = ctx.enter_context(tc.tile_pool(name="prp", bufs=3))
        smpool = ctx.enter_context(tc.tile_pool(name="smp", bufs=3))

        # --- PE pre-warm: dummy matmuls on memset scratch while the first
        # input chunk is in flight, lifting the HAM clock gate (1.2 ->
        # 2.4 GHz needs ~3.4us of sustained PE activity).  The memset runs
        # on Vector (idle early) so GpSimd can start issuing DMAs. ---
        warm_sb = wpool.tile([P, FH], BF16, tag="warm", name="warm_sb")
        nc.vector.memset(warm_sb[:], 0.0)
        warm_ps = pspool.tile([P, 2 * FH], F32, tag="ps", name="warm_ps")
        for _ in range(12):
            nc.tensor.matmul(
                warm_ps[:, 0:FH], lhsT=warm_sb[:, 0:P], rhs=warm_sb[:],
                start=True, stop=True,
            )

        # --- input DMAs.  x: per chunk-pair [128, 2048] tiles, rows
        # host-interleaved (kp, p, g) so each partition's 2 KB is one
        # contiguous DRAM run.  mt: host-repacked per o-tile so tile t's
        # weights for all chunk-pairs are one [128, 1024] contiguous
        # block.  The two HWDGE queues (Sync, Scalar) share ~270 GB/s, so
        # the interleave below lands mtt0/mtt1 + all of x first (the
        # postprocess of every tile is gated on x completing), then the
        # later tiles' mt blocks.  Params ride the GpSimd SWDGE. ---
        x_tiles = [None] * KT2
        mtt_tiles = [None] * OT

        def xdma(eng, k):
            xt = xpool.tile([P, 2 * BC * DS], FP8, tag=f"x{k}", name=f"x{k}")
            eng.dma_start(
                xt[:],
                xc[k * 2 * P:(k + 1) * 2 * P, :].rearrange(
                    "(p g) c -> p (g c)", g=2),
            )
            x_tiles[k] = xt

        def mdma(eng, t):
            mtk = xpool.tile([P, KT2 * 2 * P], FP8, tag=f"mtt{t}", name=f"mtt{t}")
            eng.dma_start(mtk[:], mt[t * P:(t + 1) * P, :])
            mtt_tiles[t] = mtk

        mdma(nc.sync, 0)
        mdma(nc.scalar, 1)
        xdma(nc.sync, 0)
        xdma(nc.scalar, 1)
        xdma(nc.sync, 2)
        xdma(nc.scalar, 3)
        mdma(nc.sync, 2)
        mdma(nc.scalar, 3)
        mdma(nc.sync, 4)
        mdma(nc.scalar, 5)
        mdma(nc.sync, 6)
        mdma(nc.scalar, 7)

        pbf_sb = wpool.tile([P, PB_COLS], BF16, tag="pbf", name="pbf_sb")
        nc.gpsimd.dma_start(pbf_sb[:], pbf[:, :])
        pf_sb = wpool.tile([P, PF_COLS], F32, tag="pf32", name="pf_sb")
        nc.gpsimd.dma_start(pf_sb[:], pf32[:, :])

        out_sb = wpool.tile([P, OT * BC], F32, tag="out", name="out_sb")

        # Pre-expanded (broadcast-over-b) parameter tiles, built once on
        # the otherwise-idle GpSimd so the per-tile bias/soma ops become
        # flat 2D/3D patterns (4D broadcast APs dispatch slowly there).
        bdend_x = wpool.tile([P, OT * BD], F32, tag="bdx", name="bdend_x")
        nc.gpsimd.tensor_copy(
            bdend_x[:].rearrange("p (t b d) -> p t b d", t=OT, d=D),
            pf_sb[:, PF_BD:PF_BD + OT * D]
            .rearrange("p (t d) -> p t d", t=OT).unsqueeze(2)
            .broadcast_to([P, OT, BC, D]),
        )
        wdend_x = wpool.tile([P, OT * BD], BF16, tag="wdx", name="wdend_x")
        nc.gpsimd.tensor_copy(
            wdend_x[:].rearrange("p (t b d) -> p t b d", t=OT, d=D),
            pbf_sb[:, PB_WD:PB_WD + OT * D]
            .rearrange("p (t d) -> p t d", t=OT).unsqueeze(2)
            .broadcast_to([P, OT, BC, D]),
        )
        bsoma_x = wpool.tile([P, OT * BC], F32, tag="bsx", name="bsoma_x")
        nc.gpsimd.tensor_copy(
            bsoma_x[:].rearrange("p (t b) -> p t b", t=OT),
            pf_sb[:, PF_BS:PF_BS + OT].unsqueeze(2)
            .broadcast_to([P, OT, BC]),
        )

        def wsyn_bf(t):
            return (pbf_sb[:, PB_WS + t * DS:PB_WS + (t + 1) * DS]
                    .unsqueeze(1).broadcast_to([P, BC, DS]))

        def postprocess_pair(te, ps_a, ps_b):
            # Two o-tiles (te, te+1) through one chain of double-width ops:
            # halves the per-op fixed cost and the cross-engine sync count.
            # Scalar: two PSUM -> SBUF bf16 copies into one [128, 2048]
            # tile; DVE: one bf16 2x w_syn multiply, s-tree, bias; GpSimd:
            # soma; Scalar: tanh + per-tile sigmoids.
            cp = prpool.tile([P, 4 * FH], BF16, tag="cp", name=f"cp{te}")
            nc.scalar.activation(cp[:, 0:2 * FH], ps_a[:], AF.Copy)
            nc.scalar.activation(cp[:, 2 * FH:4 * FH], ps_b[:], AF.Copy)
            prod = prpool.tile([P, 4 * FH], BF16, tag="prod", name=f"prod{te}")
            wsyn2 = (pbf_sb[:, PB_WS + te * DS:PB_WS + (te + 2) * DS]
                     .rearrange("p (u q) -> p u q", u=2).unsqueeze(2)
                     .broadcast_to([P, 2, BC, DS]))
            nc.vector.tensor_mul(
                prod[:].rearrange("p (u b q) -> p u b q", u=2, b=BC),
                cp[:].rearrange("p (u b q) -> p u b q", u=2, b=BC),
                wsyn2,
            )
            pv = prod[:].rearrange("p (bd s) -> p bd s", s=S)
            gr1 = smpool.tile([P, 2 * BD * 8], BF16, tag="gr1", name=f"gr1{te}")
            g1 = gr1[:].rearrange("p (bd s) -> p bd s", s=8)
            nc.vector.tensor_add(g1, pv[:, :, 0:8], pv[:, :, 8:16])
            gr2 = smpool.tile([P, 2 * BD * 4], BF16, tag="gr2", name=f"gr2{te}")
            g2 = gr2[:].rearrange("p (bd s) -> p bd s", s=4)
            nc.vector.tensor_add(g2, g1[:, :, 0:4], g1[:, :, 4:8])
            dp = smpool.tile([P, 2 * BD], F32, tag="dp", name=f"dp{te}")
            nc.vector.tensor_reduce(dp[:], g2, axis=AX.X, op=OP.add)
            nc.gpsimd.tensor_add(
                dp[:], dp[:], bdend_x[:, te * BD:(te + 2) * BD])
            dend = smpool.tile([P, 2 * BD], BF16, tag="dend", name=f"dend{te}")
            nc.scalar.activation(dend[:], dp[:], AF.Tanh)
            sp = smpool.tile([P, 2 * BD], BF16, tag="sp", name=f"sp{te}")
            nc.gpsimd.tensor_mul(
                sp[:], dend[:], wdend_x[:, te * BD:(te + 2) * BD])
            spv = sp[:].rearrange("p (ub d) -> p ub d", d=D)
            r1 = smpool.tile([P, 2 * BC * 4], BF16, tag="r1", name=f"r1{te}")
            r1v = r1[:].rearrange("p (ub d) -> p ub d", d=4)
            nc.gpsimd.tensor_add(r1v, spv[:, :, 0:4], spv[:, :, 4:8])
            r2 = smpool.tile([P, 2 * BC * 2], BF16, tag="r2", name=f"r2{te}")
            r2v = r2[:].rearrange("p (ub d) -> p ub d", d=2)
            nc.gpsimd.tensor_add(r2v, r1v[:, :, 0:2], r1v[:, :, 2:4])
            sm0 = smpool.tile([P, 2 * BC], F32, tag="sm0", name=f"sm0{te}")
            nc.gpsimd.tensor_add(
                sm0[:].unsqueeze(2), r2v[:, :, 0:1], r2v[:, :, 1:2])
            soma = smpool.tile([P, 2 * BC], F32, tag="soma", name=f"soma{te}")
            nc.gpsimd.tensor_add(
                soma[:], sm0[:], bsoma_x[:, te * BC:(te + 2) * BC])
            nc.scalar.activation(
                out_sb[:, te * BC:(te + 2) * BC], soma[:], AF.Sigmoid)

        def postprocess_single(t, pst):
            # Tail-latency variant for the last tiles: no scalar copy (DVE
            # multiplies straight from PSUM), soma + bias on DVE.
            prod = prpool.tile([P, BC * DS], BF16, tag="prods", name=f"prod{t}")
            nc.vector.tensor_mul(
                prod[:].rearrange("p (b q) -> p b q", b=BC),
                pst[:].rearrange("p (b q) -> p b q", b=BC),
                wsyn_bf(t),
            )
            pv = prod[:].rearrange("p (bd s) -> p bd s", s=S)
            gr1 = smpool.tile([P, BD * 8], BF16, tag="sg1", name=f"sg1{t}")
            g1 = gr1[:].rearrange("p (bd s) -> p bd s", s=8)
            nc.vector.tensor_add(g1, pv[:, :, 0:8], pv[:, :, 8:16])
            gr2 = smpool.tile([P, BD * 4], BF16, tag="sg2", name=f"sg2{t}")
            g2 = gr2[:].rearrange("p (bd s) -> p bd s", s=4)
            nc.vector.tensor_add(g2, g1[:, :, 0:4], g1[:, :, 4:8])
            dp = smpool.tile([P, BD], F32, tag="sdp", name=f"sdp{t}")
            nc.vector.tensor_reduce(dp[:], g2, axis=AX.X, op=OP.add)
            nc.vector.tensor_add(
                dp[:], dp[:], bdend_x[:, t * BD:(t + 1) * BD])
            dend = smpool.tile([P, BD], BF16, tag="sdd", name=f"sdd{t}")
            nc.scalar.activation(dend[:], dp[:], AF.Tanh)
            sp = smpool.tile([P, BD], BF16, tag="ssp", name=f"ssp{t}")
            nc.vector.tensor_mul(
                sp[:], dend[:], wdend_x[:, t * BD:(t + 1) * BD])
            soma = smpool.tile([P, BC], F32, tag="ssm", name=f"ssm{t}")
            nc.vector.tensor_reduce(
                soma[:], sp[:].rearrange("p (b d) -> p b d", d=D),
                axis=AX.X, op=OP.add)
            nc.scalar.activation(
                out_sb[:, t * BC:(t + 1) * BC], soma[:], AF.Sigmoid,
                bias=pf_sb[:, PF_BS + t:PF_BS + t + 1],
            )

        def mm(pst, t, k, h):
            rhs = (x_tiles[k][:].rearrange("p (g c) -> p g c", g=2)
                   [:, :, h * FH:(h + 1) * FH])
            lhs = (mtt_tiles[t][:].rearrange("p (kp g c) -> p kp g c",
                                             kp=KT2, g=2)[:, k, :, :])
            nc.tensor.matmul(
                pst[:, h * FH:(h + 1) * FH], lhsT=lhs, rhs=rhs,
                start=(k == 0), stop=(k == KT2 - 1), perf_mode=DR,
            )

        # Leading group: k-outer over o-tiles {0,1} — per-k PE work paces
        # with the x chunk-pair DMA stream.
        pst = {}
        for t in range(OT):
            pst[t] = None
        for t in range(GRP):
            pst[t] = pspool.tile([P, 2 * FH], F32, tag="ps", name=f"ps{t}")
        for k in range(KT2):
            for t in range(GRP):
                for h in range(2):
                    mm(pst[t], t, k, h)
        postprocess_pair(0, pst[0], pst[1])

        # Trailing o-tiles: k-inner; pairs {2,3} and {4,5} overlap the
        # next tiles' matmuls; tiles 6 and 7 run as latency-optimized
        # singles to keep the serial tail short.
        for t in range(GRP, OT):
            pst[t] = pspool.tile([P, 2 * FH], F32, tag="ps", name=f"ps{t}")
            for k in range(KT2):
                for h in range(2):
                    mm(pst[t], t, k, h)
            if t in (3, 5):
                postprocess_pair(t - 1, pst[t - 1], pst[t])
            elif t in (6, 7):
                postprocess_single(t, pst[t])

        nc.sync.dma_start(out[:, :], out_sb[:])

    if legalize:
        legalize_waits(nc)
    return nc


def get_nc():
    if "nc" not in _NC_CACHE:
        _NC_CACHE["nc"] = build_nc()
    return _NC_CACHE["nc"]


def pack_params(w_syn, b_dend, w_dend, b_soma):
    """Pack per-neuron parameters into the fp32 and bf16 SBUF layouts
    (each section o-tile-major: column block t holds o-tile t's rows)."""
    ws = np.asarray(w_syn, np.float32).reshape(OT, P, DS).transpose(1, 0, 2).reshape(P, OT * DS)
    bd = np.asarray(b_dend, np.float32).reshape(OT, P, D).transpose(1, 0, 2).reshape(P, OT * D)
    wd = np.asarray(w_dend, np.float32).reshape(OT, P, D).transpose(1, 0, 2).reshape(P, OT * D)
    bs = np.asarray(b_soma, np.float32).reshape(OT, P).T
    pf = np.ascontiguousarray(np.concatenate([bd, bs], axis=1))
    pb = np.ascontiguousarray(
        np.concatenate([ws, wd], axis=1).astype(ml_dtypes.bfloat16))
    return pf, pb


def interleave_rows(a):
    """Reorder [N, C] rows from (kp, g, p) to (kp, p, g) so each SBUF
    partition's DoubleRow pair is one contiguous 2C-byte DRAM run."""
    return np.ascontiguousarray(
        a.reshape(KT2, 2, P, a.shape[1]).transpose(0, 2, 1, 3)
        .reshape(N, a.shape[1]))


def prepare_in_maps(x, matriz_conexao, w_syn, b_dend, w_dend, b_soma):
    x = np.asarray(x, dtype=np.float32)
    # mt repacked per o-tile: row (t, p), cols (kp, g, c) so tile t's
    # DoubleRow weights for all chunk-pairs are one contiguous block.
    mtT = np.asarray(matriz_conexao, np.float32).T.astype(ml_dtypes.float8_e4m3)
    mt_np = np.ascontiguousarray(
        mtT.reshape(KT2, 2, P, OT, P).transpose(3, 2, 0, 1, 4).reshape(N, N))
    pf, pb = pack_params(w_syn, b_dend, w_dend, b_soma)
    xt = np.ascontiguousarray(x.transpose(1, 0, 2, 3).reshape(N, B, DS))
    in_maps = []
    for c in range(NCORES):
        xc_np = interleave_rows(
            xt[:, c * BC:(c + 1) * BC, :].reshape(N, BC * DS)
            .astype(ml_dtypes.float8_e4m3))
        in_maps.append({"mt": mt_np, "xc": xc_np, "pf32": pf, "pbf": pb})
    return in_maps


def assemble_output(results):
    outs = []
    for c in range(NCORES):
        oc = np.asarray(results[c]["out"])          # [P, OT*BC] = (oi, (t, b))
        outs.append(oc.reshape(P, OT, BC).transpose(2, 1, 0).reshape(BC, N))
    return np.ascontiguousarray(np.concatenate(outs, axis=0).astype(np.float32))


def kernel(x, matriz_conexao, w_syn, b_dend, w_dend, b_soma):
    from concourse.bass_utils import run_bass_kernel_spmd
    in_maps = prepare_in_maps(x, matriz_conexao, w_syn, b_dend, w_dend, b_soma)
    nc = get_nc()
    res = run_bass_kernel_spmd(nc, in_maps, list(range(NCORES)))
    return assemble_output(res.results)
